# revision 1
# baseline (speedup 1.0000x reference)
"""CLIP (InfoNCE) loss kernel for Trainium2, 8 NeuronCores.

loss = 0.5*(ce_m + ce_s) where
  ce_m = mean_i( LSE_j(l[i,:]) - l[i,i] ),  ce_s = mean_j( LSE_i(l[:,j]) - l[j,j] )
  l = logit_scale * (m @ s.T),  B=16384, D=256.

Strategy (data parallel on batch rows, 8 cores):
  - core c owns rows [c*2048, (c+1)*2048) of m; gets the FULL s.
  - host pre-scales m by logit_scale and pre-transposes both operands to
    [D, rows] bf16 so they feed the PE directly (lhsT / rhs layout).
  - per core, 16 m-tiles x 16 column-groups of [128, 1024] logits in PSUM
    (f32 accumulation over K=256 in two 128-chunks, 4 matmuls per group).
  - one ScalarE activation per group computes E = exp(l - SHIFT) (bf16);
    for half the groups the fused accum_out also emits the per-row partial
    sum (f32, on the un-rounded values); for the other half a DVE reduce
    of E does it — balancing ScalarE vs VectorE.
  - per-column sums of E accumulate in PSUM via ones-vector matmuls, four
    N=256 matmuls per group aimed at four different 32-column PE strips
    (tile_position) of ONE psum bank, so they execute concurrently.
  - diag l[i,i] is computed exactly in f32 as a row-dot of the
    natural-layout shards (DVE mul+reduce).
  - host merges the tiny per-core partials in float64:
      rowLSE = SHIFT + log(rowsum); colLSE = SHIFT + log(sum_c colsum_c)
      loss = mean(0.5*(rowLSE + colLSE) - diag)

SHIFT is a single global logsumexp shift. logits ~ N(0, (scale*sqrt(D))^2);
their max over B^2 samples is ~6 sigma, so SHIFT = 6*|scale|*sqrt(D) keeps
exp() in-range: no overflow, and anything that underflows f32 is ~e^-80
below the column max — far below f32 relative precision anyway.
"""

import math
from contextlib import ExitStack

import numpy as np
import ml_dtypes

import concourse.bacc as bacc
import concourse.bass as bass
import concourse.tile as tile
from concourse import mybir
from concourse.bass_utils import run_bass_kernel_spmd

BF16 = ml_dtypes.bfloat16

B = 16384
D = 256
NCORES = 8
ROWS = B // NCORES          # 2048 rows per core
P = 128
MT = ROWS // P              # 16 m-tiles
PN = 512                    # psum bank width (f32)
GW = 2                      # panels per exp-group -> [128, 1024] ACT ops
GN = B // (PN * GW)         # 16 column-groups
KC = D // P                 # 2 contraction chunks
NQ = 8                      # sT DMA chunks per k (early-start pipelining)
QW = B // NQ
CS = 2                      # column-sum split per panel (4 PE strips total)

f32 = mybir.dt.float32
bf16 = mybir.dt.bfloat16

_nc_cache: dict[float, "bass.Bass"] = {}


def _build(shift: float) -> "bass.Bass":
    nc = bacc.Bacc(trn_type="TRN2")

    mT_d = nc.dram_tensor("mT", [D, ROWS], bf16, kind="ExternalInput")
    sT_d = nc.dram_tensor("sT", [D, B], bf16, kind="ExternalInput")
    mnat_d = nc.dram_tensor("mnat", [ROWS, D], f32, kind="ExternalInput")
    snat_d = nc.dram_tensor("snat", [ROWS, D], f32, kind="ExternalInput")

    rowsum_d = nc.dram_tensor("rowsum", [P, MT], f32, kind="ExternalOutput")
    diag_d = nc.dram_tensor("diag", [P, MT], f32, kind="ExternalOutput")
    colsum_d = nc.dram_tensor("colsum", [GW, GN * PN], f32, kind="ExternalOutput")

    nstrips = GW * CS
    w = PN // CS

    with ExitStack() as ctx:
        tc = ctx.enter_context(tile.TileContext(nc))
        singles = ctx.enter_context(tc.tile_pool(name="singles", bufs=1))
        # bufs=10 measured ~45us faster than 6: deeper E buffering lets the
        # column-sum matmuls and copies lag behind ScalarE without stalling it
        epool = ctx.enter_context(tc.tile_pool(name="epool", bufs=10))
        diagpool = ctx.enter_context(tc.tile_pool(name="diagpool", bufs=4))
        mainps = ctx.enter_context(tc.tile_pool(name="mainps", bufs=3, space="PSUM"))
        colps = ctx.enter_context(tc.tile_pool(name="colps", bufs=2, space="PSUM"))

        mT_sb = singles.tile([P, KC, ROWS], bf16, tag="mT")
        for k in range(KC):
            nc.sync.dma_start(out=mT_sb[:, k, :], in_=mT_d[k * P : (k + 1) * P, :])
        sT_sb = [
            [
                singles.tile([P, QW], bf16, name=f"sT_{k}_{q}", tag=f"sT_{k}_{q}")
                for q in range(NQ)
            ]
            for k in range(KC)
        ]
        # q-major order so the first column-group's two k-chunks land first
        for q in range(NQ):
            for k in range(KC):
                nc.sync.dma_start(
                    out=sT_sb[k][q],
                    in_=sT_d[k * P : (k + 1) * P, q * QW : (q + 1) * QW],
                )

        ones = singles.tile([P, 1], bf16, tag="ones")
        nc.vector.memset(ones, 1.0)
        negshift = singles.tile([P, 1], f32, tag="negshift")
        nc.vector.memset(negshift, -shift)

        rowsums_sb = singles.tile([P, MT * GN], f32, tag="rowsums")
        rowfinal = singles.tile([P, MT], f32, tag="rowfinal")
        colsum_sb = [
            singles.tile([1, GN * PN], f32, name=f"colsum_{i}", tag=f"colsum_{i}")
            for i in range(GW)
        ]
        diagfinal = singles.tile([P, MT], f32, tag="diagfinal")

        for g in range(GN):
            colpsum = colps.tile([32 * (nstrips - 1) + 1, PN], f32)  # one bank
            for mt in range(MT):
                ps = mainps.tile([P, GW * PN], f32)  # 2 banks
                for k in range(KC):
                    for sub in range(GW):
                        nt = g * GW + sub
                        q, j = divmod(nt, QW // PN)
                        nc.tensor.matmul(
                            ps[:, sub * PN : (sub + 1) * PN],
                            lhsT=mT_sb[:, k, mt * P : (mt + 1) * P],
                            rhs=sT_sb[k][q][:, j * PN : (j + 1) * PN],
                            start=(k == 0),
                            stop=(k == KC - 1),
                        )
                e = epool.tile([P, GW * PN], bf16)
                slot = mt * GN + g
                if (mt + g) % 2 == 0:
                    nc.scalar.activation(
                        e, ps, mybir.ActivationFunctionType.Exp,
                        bias=negshift[:, 0:1], scale=1.0,
                        accum_out=rowsums_sb[:, slot : slot + 1],
                    )
                else:
                    nc.scalar.activation(
                        e, ps, mybir.ActivationFunctionType.Exp,
                        bias=negshift[:, 0:1], scale=1.0,
                    )
                    nc.vector.reduce_sum(
                        rowsums_sb[:, slot : slot + 1], e,
                        axis=mybir.AxisListType.X,
                    )
                for sub in range(GW):
                    for ci in range(CS):
                        strip = sub * CS + ci
                        nc.tensor.matmul(
                            colpsum[32 * strip : 32 * strip + 1, 0:w],
                            lhsT=ones,
                            rhs=e[:, sub * PN + ci * w : sub * PN + (ci + 1) * w],
                            start=(mt == 0),
                            stop=(mt == MT - 1),
                            tile_position=(0, 32 * strip),
                        )
            for sub in range(GW):
                for ci in range(CS):
                    strip = sub * CS + ci
                    nc.vector.tensor_copy(
                        out=colsum_sb[sub][
                            :, g * PN + ci * w : g * PN + (ci + 1) * w
                        ],
                        in_=colpsum[32 * strip : 32 * strip + 1, 0:w],
                    )

        # diag + final row reduction emitted last (lowest scheduler priority;
        # DVE/DMA fill gaps while PE/ACT stream)
        for mt in range(MT):
            mn = diagpool.tile([P, D], f32, tag="mn")
            sn = diagpool.tile([P, D], f32, tag="sn")
            prod = diagpool.tile([P, D], f32, tag="prod")
            nc.sync.dma_start(out=mn, in_=mnat_d[mt * P : (mt + 1) * P, :])
            nc.sync.dma_start(out=sn, in_=snat_d[mt * P : (mt + 1) * P, :])
            # (tensor_tensor_reduce compiles but faults on this HW/runtime
            # combo — use plain mul + reduce instead)
            nc.vector.tensor_mul(prod, mn, sn)
            nc.vector.reduce_sum(
                diagfinal[:, mt : mt + 1], prod, axis=mybir.AxisListType.X
            )

        for mt in range(MT):
            nc.vector.reduce_sum(
                rowfinal[:, mt : mt + 1],
                rowsums_sb[:, mt * GN : (mt + 1) * GN],
                axis=mybir.AxisListType.X,
            )

        nc.sync.dma_start(out=rowsum_d[:, :], in_=rowfinal)
        nc.sync.dma_start(out=diag_d[:, :], in_=diagfinal)
        for sub in range(GW):
            nc.sync.dma_start(out=colsum_d[sub : sub + 1, :], in_=colsum_sb[sub])

    nc.compile()
    return nc


def _get_nc(shift: float) -> "bass.Bass":
    if shift not in _nc_cache:
        _nc_cache[shift] = _build(shift)
    return _nc_cache[shift]


def run(inputs: dict, trace: bool = False):
    m = np.asarray(inputs["modality_features"], dtype=np.float32)
    s = np.asarray(inputs["sequence_features"], dtype=np.float32)
    scale = float(np.asarray(inputs["logit_scale"], dtype=np.float32))
    assert m.shape == (B, D) and s.shape == (B, D)

    shift = float(6.0 * abs(scale) * math.sqrt(D))
    nc = _get_nc(shift)

    ms = m * np.float32(scale)
    sT_full = np.ascontiguousarray(s.T).astype(BF16)

    in_maps = []
    for c in range(NCORES):
        r = slice(c * ROWS, (c + 1) * ROWS)
        in_maps.append(
            {
                "mT": np.ascontiguousarray(ms[r].T).astype(BF16),
                "sT": sT_full,
                "mnat": np.ascontiguousarray(ms[r]),
                "snat": np.ascontiguousarray(s[r]),
            }
        )

    res = run_bass_kernel_spmd(nc, in_maps, list(range(NCORES)), trace=trace)

    rowsum = np.concatenate(
        [r["rowsum"].T.reshape(-1) for r in res.results]
    ).astype(np.float64)
    diag = np.concatenate([r["diag"].T.reshape(-1) for r in res.results]).astype(
        np.float64
    )
    colsum = np.zeros(B, dtype=np.float64)
    for r in res.results:
        # colsum_d[sub, g*PN + j] holds column g*(GW*PN) + sub*PN + j
        arr = r["colsum"].astype(np.float64)
        colsum += arr.reshape(GW, GN, PN).transpose(1, 0, 2).reshape(B)

    rowlse = shift + np.log(rowsum)
    collse = shift + np.log(colsum)
    loss = np.mean(0.5 * (rowlse + collse) - diag)
    return np.asarray(loss, dtype=np.float32), res


def kernel(**inputs) -> np.ndarray:
    out, _ = run(inputs, trace=False)
    return out



# revision 2
# speedup vs baseline: 1.2304x; 1.2304x over previous
"""CLIP (InfoNCE) loss kernel for Trainium2, 8 NeuronCores.

loss = 0.5*(ce_m + ce_s) where
  ce_m = mean_i( LSE_j(l[i,:]) - l[i,i] ),  ce_s = mean_j( LSE_i(l[:,j]) - l[j,j] )
  l = logit_scale * (m @ s.T),  B=16384, D=256.

Data parallel on batch rows, 8 cores; core c owns rows [c*2048, (c+1)*2048)
of m and sees the full s.

Per core:
  - Features are quantized (UNSCALED) to fp8 e4m3 in a k-interleaved layout
    [128, 2, N]; main logits tiles use DoubleRow fp8 matmuls (K=256 fused in
    one PE pass, ~2x bf16 FLOPs).  logit_scale is applied inside the ACT
    affine (exp(scale*l - shift)), so quantization never clips.
  - mt-outer / g-inner tiling: 16 row-tiles x 8 column groups of [128, 2048].
    One ScalarE exp per group tile (PSUM f32 -> SBUF bf16) with fused
    accum_out producing the per-row partial sums (computed pre-rounding in
    f32) -- ScalarE is the bottleneck engine and runs ~94% occupied.
  - Column sums: per-group bf16 accumulators acc_g += E on DVE (idle
    otherwise); at mt=0 ACT writes acc_g directly.  After the last row-tile,
    ones-vector matmuls reduce each acc_g across partitions, batched 3 groups
    per borrowed PSUM slot on PE column strips 0/32/64 (quad 3 unusable).
  - diag l[i,i] is exact f32: row-dot of natural-layout scaled shards
    (DVE mul+reduce), spread through the main loop.
  - host merges per-core partials in float64:
      rowLSE = SHIFT + log(rowsum); colLSE = SHIFT + log(sum_c colsum_c)
      loss = mean(0.5*(rowLSE + colLSE) - diag)

SHIFT = 6*|scale|*sqrt(D) (a ~6-sigma bound on logits ~ N(0, scale^2 D)):
exp never overflows, and underflow to 0 only hits terms ~e^-80 below the
row/col max -- far below f32 relative precision.  fp8 quantization of the
inputs perturbs the loss by ~7e-4 relative (tolerance 2e-2): LSE is
max-dominated, the exact-diag term is computed in f32, and quantization
noise on 256-term dots is ~0.5 absolute on logits with sigma=16.
"""

import math
from contextlib import ExitStack

import numpy as np
import ml_dtypes

import concourse.bacc as bacc
import concourse.tile as tile
from concourse import mybir
from concourse.bass_utils import run_bass_kernel_spmd

FP8 = ml_dtypes.float8_e4m3

B = 16384
D = 256
NCORES = 8
ROWS = B // NCORES          # 2048 rows per core
P = 128
MT = ROWS // P              # 16 row-tiles
KC = D // P                 # 2 k-chunks (fused by DoubleRow)
W = 2048                    # column group width (4 psum banks f32)
GN = B // W                 # 8 column groups
SUBW = 512                  # matmul free dim (one psum bank)
NSUB = W // SUBW            # 4

f32 = mybir.dt.float32
bf16 = mybir.dt.bfloat16
fp8 = mybir.dt.float8e4

_nc_cache: dict = {}


def _build(shift: float, scale: float) -> "bacc.Bacc":
    nc = bacc.Bacc(trn_type="TRN2")

    m8_d = nc.dram_tensor("m8", [P, KC, ROWS], fp8, kind="ExternalInput")
    s8_d = nc.dram_tensor("s8", [P, KC, B], fp8, kind="ExternalInput")
    mnat_d = nc.dram_tensor("mnat", [ROWS, D], f32, kind="ExternalInput")
    snat_d = nc.dram_tensor("snat", [ROWS, D], f32, kind="ExternalInput")

    rowsum_d = nc.dram_tensor("rowsum", [P, MT], f32, kind="ExternalOutput")
    diag_d = nc.dram_tensor("diag", [P, MT], f32, kind="ExternalOutput")
    colsum_d = nc.dram_tensor("colsum", [GN, W], f32, kind="ExternalOutput")

    with ExitStack() as ctx:
        tc = ctx.enter_context(tile.TileContext(nc))
        singles = ctx.enter_context(tc.tile_pool(name="singles", bufs=1))
        epool = ctx.enter_context(tc.tile_pool(name="epool", bufs=6))
        diagpool = ctx.enter_context(tc.tile_pool(name="diagpool", bufs=4))
        mainps = ctx.enter_context(tc.tile_pool(name="mainps", bufs=2, space="PSUM"))

        # ramp: first row-tile of m8 and first chunk of s8[0] land first
        m8_sb = singles.tile([P, KC, ROWS], fp8, tag="m8")
        nc.sync.dma_start(out=m8_sb[:, :, 0:P], in_=m8_d[:, :, 0:P])
        s8_sb = [
            singles.tile([P, KC, W], fp8, name=f"s8_{g}", tag=f"s8_{g}")
            for g in range(GN)
        ]
        for q in range(4):
            nc.sync.dma_start(
                out=s8_sb[0][:, :, q * SUBW : (q + 1) * SUBW],
                in_=s8_d[:, :, q * SUBW : (q + 1) * SUBW],
            )
        nc.sync.dma_start(out=m8_sb[:, :, P:ROWS], in_=m8_d[:, :, P:ROWS])
        for g in range(1, GN):
            nc.sync.dma_start(out=s8_sb[g], in_=s8_d[:, :, g * W : (g + 1) * W])

        ones = singles.tile([P, 1], bf16, tag="ones")
        nc.vector.memset(ones, 1.0)
        negshift = singles.tile([P, 1], f32, tag="negshift")
        nc.vector.memset(negshift, -shift)

        rowsums_sb = singles.tile([P, MT * GN], f32, tag="rowsums")
        rowfinal = singles.tile([P, MT], f32, tag="rowfinal")
        colsum_sb = singles.tile([P, 3 * W], f32, tag="colsum")
        diagfinal = singles.tile([P, MT], f32, tag="diagfinal")
        accs = [
            singles.tile([P, W], bf16, name=f"acc_{g}", tag=f"acc_{g}")
            for g in range(GN)
        ]

        for mt in range(MT):
            for g in range(GN):
                ps = mainps.tile([P, W], f32, tag="ps")
                for sub in range(NSUB):
                    nc.tensor.matmul(
                        ps[:, sub * SUBW : (sub + 1) * SUBW],
                        lhsT=m8_sb[:, :, mt * P : (mt + 1) * P],
                        rhs=s8_sb[g][:, :, sub * SUBW : (sub + 1) * SUBW],
                        start=True,
                        stop=True,
                        perf_mode=mybir.MatmulPerfMode.DoubleRow,
                    )
                slot = mt * GN + g
                if mt == 0:
                    nc.scalar.activation(
                        accs[g], ps, mybir.ActivationFunctionType.Exp,
                        bias=negshift[:, 0:1], scale=scale,
                        accum_out=rowsums_sb[:, slot : slot + 1],
                    )
                else:
                    e = epool.tile([P, W], bf16)
                    nc.scalar.activation(
                        e, ps, mybir.ActivationFunctionType.Exp,
                        bias=negshift[:, 0:1], scale=scale,
                        accum_out=rowsums_sb[:, slot : slot + 1],
                    )
                    nc.vector.tensor_add(accs[g], e, accs[g])
                if mt == MT - 1 and g in (2, 5, 7):
                    # colsum batch: acc_g final after add(15, g); borrow a
                    # mainps slot, one PE column strip (0/32/64) per group
                    batch = {2: (0, 1, 2), 5: (3, 4, 5), 7: (6, 7)}[g]
                    b = {2: 0, 5: 1, 7: 2}[g]
                    colps = mainps.tile([P, W], f32, tag="ps", name="colps")
                    for idx, gg in enumerate(batch):
                        row = 32 * idx
                        for sub in range(NSUB):
                            nc.tensor.matmul(
                                colps[row : row + 1, sub * SUBW : (sub + 1) * SUBW],
                                lhsT=ones,
                                rhs=accs[gg][:, sub * SUBW : (sub + 1) * SUBW],
                                start=True,
                                stop=True,
                            )
                    nb = len(batch)
                    hi = 32 * (nb - 1) + 1
                    nc.vector.tensor_copy(
                        out=colsum_sb[0:hi, b * W : (b + 1) * W],
                        in_=colps[0:hi, :],
                    )
                    nc.sync.dma_start(
                        out=colsum_d[3 * b : 3 * b + nb, :],
                        in_=colsum_sb[0 : hi : 32, b * W : (b + 1) * W],
                    )
            nc.vector.reduce_sum(
                rowfinal[:, mt : mt + 1],
                rowsums_sb[:, mt * GN : (mt + 1) * GN],
                axis=mybir.AxisListType.X,
            )
            # diag work spread through the main loop (fills DVE idle time)
            mn = diagpool.tile([P, D], f32, tag="mn")
            sn = diagpool.tile([P, D], f32, tag="sn")
            prod = diagpool.tile([P, D], f32, tag="prod")
            nc.sync.dma_start(out=mn, in_=mnat_d[mt * P : (mt + 1) * P, :])
            nc.sync.dma_start(out=sn, in_=snat_d[mt * P : (mt + 1) * P, :])
            nc.vector.tensor_mul(prod, mn, sn)
            nc.vector.reduce_sum(
                diagfinal[:, mt : mt + 1], prod, axis=mybir.AxisListType.X
            )

        nc.sync.dma_start(out=rowsum_d[:, :], in_=rowfinal)
        nc.sync.dma_start(out=diag_d[:, :], in_=diagfinal)

    nc.compile()
    return nc


def _get_nc(shift: float, scale: float):
    key = (shift, scale)
    if key not in _nc_cache:
        _nc_cache[key] = _build(shift, scale)
    return _nc_cache[key]


def _interleave_fp8(x: np.ndarray) -> np.ndarray:
    """x [N, D] f32 -> [P, KC, N] fp8 with x8[p, j, c] = x[c, j*128 + p]."""
    xq = x.astype(FP8)
    xT = np.ascontiguousarray(xq.T)  # [D, N]
    return np.ascontiguousarray(xT.reshape(KC, P, -1).transpose(1, 0, 2))


def run(inputs: dict, trace: bool = False):
    m = np.asarray(inputs["modality_features"], dtype=np.float32)
    s = np.asarray(inputs["sequence_features"], dtype=np.float32)
    scale = float(np.asarray(inputs["logit_scale"], dtype=np.float32))
    assert m.shape == (B, D) and s.shape == (B, D)

    shift = float(6.0 * abs(scale) * math.sqrt(D))
    nc = _get_nc(shift, scale)

    ms = m * np.float32(scale)
    s8 = _interleave_fp8(s)

    in_maps = []
    for c in range(NCORES):
        r = slice(c * ROWS, (c + 1) * ROWS)
        in_maps.append(
            {
                "m8": _interleave_fp8(m[r]),
                "s8": s8,
                "mnat": np.ascontiguousarray(ms[r]),
                "snat": np.ascontiguousarray(s[r]),
            }
        )

    res = run_bass_kernel_spmd(nc, in_maps, list(range(NCORES)), trace=trace)

    rowsum = np.concatenate(
        [r["rowsum"].T.reshape(-1) for r in res.results]
    ).astype(np.float64)
    diag = np.concatenate([r["diag"].T.reshape(-1) for r in res.results]).astype(
        np.float64
    )
    colsum = np.zeros(B, dtype=np.float64)
    for r in res.results:
        colsum += r["colsum"].astype(np.float64).reshape(B)

    rowlse = shift + np.log(rowsum)
    collse = shift + np.log(colsum)
    loss = np.mean(0.5 * (rowlse + collse) - diag)
    return np.asarray(loss, dtype=np.float32), res


def kernel(**inputs) -> np.ndarray:
    out, _ = run(inputs, trace=False)
    return out
